# revision 30
# baseline (speedup 1.0000x reference)
"""MultiHeadAttention Trainium2 kernel (8 NeuronCores).

Reference computation (torch-style Linear, x @ W.T):
    k = key @ W_k.T; v = value @ W_v.T; q = query (no projection)
    scores = q @ k.T / sqrt(64) per head; attn = softmax(scores)
    out = (attn @ v) @ W_o.T

Sharding: core = (batch b, head-group g) with b in {0,1}, g in {0..3};
each core owns 4 heads of one batch. Projection weights are column-split
by head so K/V projections and attention stay core-local; the final W_o
matmul is computed as a partial sum over the core's 256 head-channels and
the 4 partials per batch are summed on host.

On-device dataflow per core (all matmuls float32r, full PE rate):
    kT[256,4096]  = W_kT.T @ keyT          (contraction over embed)
    v[4096,256]   = valueT.T @ W_vT        (+ ones column per head)
    scoresT[t,q]  = kT_h.T @ qT_h          (K=64; two heads run concurrently
                                            via tile_position rows 0/64)
    expT          = exp(scoresT / 8)       (ScalarE, from PSUM)
    outT[65,q]    = v_ext_h.T @ expT       (accumulated over 32 t-chunks;
                                            row 64 = softmax denominator)
    norm          = outT[0:64] * recip(outT[64])
    out_partial   = norm_heads.T @ W_oT    (accumulated over 4 heads)

Two-phase PSUM schedule: while the K/V stream + projections run (DMA-bound,
~93us), TWO attention sweeps (q-tile 0, both head pairs) run concurrently
from a shared 6-slot PSUM pool with single-bank score tiles, keeping ScalarE
fed. After the stream, pools are re-opened and the remaining 6 sweeps run
with double-buffered [128,1024] score tiles (higher exp efficiency).
"""

import os
import numpy as np

import concourse.bacc as bacc
import concourse.tile as tile
import concourse.mybir as mybir
from concourse.bass_utils import run_bass_kernel_spmd

F32 = mybir.dt.float32
F32R = mybir.dt.float32r
EXPF = mybir.ActivationFunctionType.Exp

B, NQ, NK, E, H, D = 2, 2048, 4096, 1024, 16, 64
HPC = 4          # heads per core
C = HPC * D      # head-channels per core (256)
TB = 256         # token block for streaming K/V projections
NTB = NK // TB   # 16
TCH = NK // 128  # 32 t-chunks for attention
QT = 512         # q tile
NJ = NQ // QT    # 4

_last_results = None
_last_in_maps = None


def _build():
    nc = bacc.Bacc("TRN2", target_bir_lowering=False, debug=False, num_devices=8)

    keyT_d = nc.dram_tensor("keyT", [E, NK], F32, kind="ExternalInput").ap()
    valT_d = nc.dram_tensor("valT", [E, NK], F32, kind="ExternalInput").ap()
    qT_d = nc.dram_tensor("qT", [C, NQ], F32, kind="ExternalInput").ap()
    wkT_d = nc.dram_tensor("wkT", [E, C], F32, kind="ExternalInput").ap()
    wvT_d = nc.dram_tensor("wvT", [E, C], F32, kind="ExternalInput").ap()
    woT_d = nc.dram_tensor("woT", [D, HPC, E], F32, kind="ExternalInput").ap()
    out_d = nc.dram_tensor("out", [NQ, E], F32, kind="ExternalOutput").ap()

    with tile.TileContext(nc) as tc:
        with (
            tc.tile_pool(name="wpool", bufs=1) as wpool,
            tc.tile_pool(name="stream", bufs=3) as stream,
            tc.tile_pool(name="big", bufs=1) as big,
            tc.tile_pool(name="expp", bufs=3) as expp,
            tc.tile_pool(name="epil", bufs=2) as epil,
            tc.tile_pool(name="normp", bufs=5) as normp,
            tc.tile_pool(name="outsb", bufs=2) as outsb,
        ):
            # ---- resident weights / q ----
            wk_sb = wpool.tile([128, 8, C], F32R)
            wv_sb = wpool.tile([128, 8, C], F32R)
            wo_sb = wpool.tile([D, HPC, E], F32R)
            q_sb = wpool.tile([128, 2, NQ], F32R)
            nc.sync.dma_start(wk_sb[:], wkT_d.rearrange("(c p) n -> p c n", p=128).bitcast(F32R))

            # ---- resident kT / v_ext ----
            kT_sb = big.tile([128, 2, NK], F32R)            # [hd%128, hd//128, t]
            vx_sb = big.tile([128, TCH, HPC, D + 1], F32R)  # [t%128, t//128, h, d|1]
            for t in range(TCH):
                nc.vector.memset(vx_sb[:, t, :, D:D + 1].bitcast(F32), 1.0)

            def emit_scores_pair(sdst_a, sdst_b, pr, t, q0):
                nc.tensor.matmul(sdst_a,
                                 kT_sb[0:64, pr, t * 128:(t + 1) * 128],
                                 q_sb[0:64, pr, q0:q0 + QT],
                                 start=True, stop=True, tile_position=(0, 0))
                nc.tensor.matmul(sdst_b,
                                 kT_sb[64:128, pr, t * 128:(t + 1) * 128],
                                 q_sb[64:128, pr, q0:q0 + QT],
                                 start=True, stop=True, tile_position=(64, 0))

            def emit_attnv(oA, oB, ex, pr, t):
                hA, hB = 2 * pr, 2 * pr + 1
                nc.tensor.matmul(oA[:], vx_sb[:, t, hA, :], ex[:, 0:QT],
                                 start=(t == 0), stop=(t == TCH - 1))
                nc.tensor.matmul(oB[:], vx_sb[:, t, hB, :], ex[:, QT:2 * QT],
                                 start=(t == 0), stop=(t == TCH - 1))

            def emit_epilogue(oT, norm_tiles):
                ocp = epil.tile([D + 1, QT], F32, tag="ocp", bufs=3, name="ocp")
                nc.vector.tensor_copy(ocp[:], oT[:])
                rc = epil.tile([1, QT], F32R, tag="recip", name="rc")
                with nc.allow_low_precision(reason="f32r recip, ~19-bit mantissa is ample"):
                    nc.vector.reciprocal(rc[0:1, :], ocp[64:65, :])
                bc = epil.tile([64, QT], F32R, tag="bcast", name="bc")
                nc.gpsimd.partition_broadcast(bc[:], rc[0:1, :])
                nm = normp.tile([64, QT], F32R, tag="norm", name="nm")
                nc.vector.tensor_mul(nm[:], ocp[0:64, :], bc[:])
                norm_tiles.append(nm)

            def emit_wo(j, norm_tiles, psw, heads=tuple(range(HPC)), accum=False,
                        wps_tag="wps"):
                # W_o partial over the given heads; accum=True accumulates
                # into the DRAM rows via SWDGE instead of overwriting.
                q0 = j * QT
                for mc in range(4):
                    osb = outsb.tile([128, E], F32, tag="osb", name="osb")
                    for et in range(2):
                        wps = psw.tile([128, QT], F32, tag=wps_tag, name="wps")
                        for i, h in enumerate(heads):
                            nc.tensor.matmul(wps[:],
                                             norm_tiles[h][:, mc * 128:(mc + 1) * 128],
                                             wo_sb[:, h, et * QT:(et + 1) * QT],
                                             start=(i == 0), stop=(i == len(heads) - 1))
                        nc.vector.tensor_copy(osb[:, et * QT:(et + 1) * QT], wps[:])
                    dst = out_d[q0 + mc * 128:q0 + (mc + 1) * 128, :]
                    if accum:
                        nc.gpsimd.dma_start(dst, osb[:], accum_op=mybir.AluOpType.add)
                    else:
                        nc.sync.dma_start(dst, osb[:])

            # ================= PHASE 1: stream + projections + j0 =================
            norm_j0 = []
            with (
                tc.tile_pool(name="pm6", bufs=6, space="PSUM") as pm6,
                tc.tile_pool(name="ps2", bufs=2, space="PSUM") as ps2,
            ):
                # two concurrent sweeps: j=0, both pairs, single-bank score tiles
                o_acc = {}
                for pr in range(2):
                    o_acc[pr] = (pm6.tile([D + 1, QT], F32, tag="p1", name="oA"),
                                 pm6.tile([D + 1, QT], F32, tag="p1", name="oB"))

                # K/V projections streamed over t blocks, interleaved with the
                # two j0 sweeps chasing the stream
                for tb in range(NTB):
                    kblk = stream.tile([128, 8, TB], F32R, tag="kblk", name="kblk")
                    vblk = stream.tile([128, 8, TB], F32R, tag="vblk", name="vblk")
                    ts0 = tb * TB
                    nc.sync.dma_start(kblk[:], keyT_d.rearrange("(c p) t -> p c t", p=128)[:, :, ts0:ts0 + TB].bitcast(F32R))
                    if tb == 0:
                        # critical-path loads for the first scores/exp: q's j0
                        # slice, then V-side weights; the rest of q after.
                        nc.sync.dma_start(q_sb[:, :, 0:QT], qT_d.rearrange("(c p) n -> p c n", p=128)[:, :, 0:QT].bitcast(F32R))
                    nc.sync.dma_start(vblk[:], valT_d.rearrange("(c p) t -> p c t", p=128)[:, :, ts0:ts0 + TB].bitcast(F32R))
                    if tb == 0:
                        nc.sync.dma_start(wv_sb[:], wvT_d.rearrange("(c p) n -> p c n", p=128).bitcast(F32R))
                        nc.sync.dma_start(q_sb[:, :, QT:NQ], qT_d.rearrange("(c p) n -> p c n", p=128)[:, :, QT:NQ].bitcast(F32R))
                    for mc in range(2):
                        kps = pm6.tile([128, TB], F32, tag="p1", name="kps")
                        for c in range(8):
                            nc.tensor.matmul(kps[:], wk_sb[:, c, mc * 128:(mc + 1) * 128],
                                             kblk[:, c, :], start=(c == 0), stop=(c == 7))
                        nc.vector.tensor_copy(kT_sb[:, mc, ts0:ts0 + TB], kps[:])
                    for t2 in range(TB // 128):
                        vps = pm6.tile([128, C], F32, tag="p1", name="vps")
                        for c in range(8):
                            nc.tensor.matmul(vps[:], vblk[:, c, t2 * 128:(t2 + 1) * 128],
                                             wv_sb[:, c, :], start=(c == 0), stop=(c == 7))
                        tg = tb * (TB // 128) + t2
                        nc.vector.tensor_copy(
                            vx_sb[:, tg, :, 0:D],
                            vps[:].rearrange("p (h d) -> p h d", h=HPC))
                    # j0 sweeps for the two t-chunks this block just produced
                    for t in (2 * tb, 2 * tb + 1):
                        for pr in range(2):
                            s1 = ps2.tile([128, QT], F32, tag="ssc", name="s1")
                            s2 = ps2.tile([128, QT], F32, tag="ssc", name="s2")
                            emit_scores_pair(s1[:], s2[:], pr, t, 0)
                            ex = expp.tile([128, 2 * QT], F32R, tag="exp", name="ex")
                            nc.scalar.activation(ex[:, 0:QT], s1[:], EXPF, scale=0.125)
                            nc.scalar.activation(ex[:, QT:2 * QT], s2[:], EXPF, scale=0.125)
                            emit_attnv(o_acc[pr][0], o_acc[pr][1], ex, pr, t)
                nc.sync.dma_start(wo_sb[:], woT_d[:].bitcast(F32R))
                for pr in range(2):
                    emit_epilogue(o_acc[pr][0], norm_j0)
                    emit_epilogue(o_acc[pr][1], norm_j0)

            # ================= PHASE 2: j1..j3 + all W_o =================
            with (
                tc.tile_pool(name="pscore", bufs=2, space="PSUM") as pscore,
                tc.tile_pool(name="pout", bufs=3, space="PSUM") as pout,
                tc.tile_pool(name="psw", bufs=1, space="PSUM") as psw,
            ):
                emit_wo(0, norm_j0, psw)
                for j in range(1, NJ):
                    q0 = j * QT
                    norm_tiles = []
                    for pr in range(2):
                        oA = pout.tile([D + 1, QT], F32, tag="outp", name="oA")
                        oB = pout.tile([D + 1, QT], F32, tag="outp", name="oB")
                        for t in range(TCH):
                            sc = pscore.tile([128, 2 * QT], F32, tag="score", name="sc")
                            emit_scores_pair(sc[:, 0:QT], sc[:, QT:2 * QT], pr, t, q0)
                            ex = expp.tile([128, 2 * QT], F32R, tag="exp", name="ex")
                            nc.scalar.activation(ex[:], sc[:], EXPF, scale=0.125)
                            emit_attnv(oA, oB, ex, pr, t)
                        emit_epilogue(oA, norm_tiles)
                        emit_epilogue(oB, norm_tiles)
                        if j == NJ - 1 and pr == 0:
                            # last q-tile: flush pair 0's W_o early so only
                            # pair 1's half remains after the final sweep
                            emit_wo(j, norm_tiles, psw, heads=(0, 1))
                    if j == NJ - 1:
                        emit_wo(j, norm_tiles, psw, heads=(2, 3), accum=True)
                    else:
                        emit_wo(j, norm_tiles, psw)

    nc.compile()
    return nc


_nc = None


def kernel(query, key, value, W_k, W_v, W_o):
    global _nc, _last_results, _last_in_maps
    if _nc is None:
        _nc = _build()

    query = np.asarray(query, dtype=np.float32)
    key = np.asarray(key, dtype=np.float32)
    value = np.asarray(value, dtype=np.float32)
    W_k = np.asarray(W_k, dtype=np.float32)
    W_v = np.asarray(W_v, dtype=np.float32)
    W_o = np.asarray(W_o, dtype=np.float32)

    keyT = [np.ascontiguousarray(key[b].T) for b in range(B)]
    valT = [np.ascontiguousarray(value[b].T) for b in range(B)]

    in_maps = []
    for b in range(B):
        for g in range(4):
            c0 = g * C
            woT = np.ascontiguousarray(
                W_o[:, c0:c0 + C].T.reshape(HPC, D, E).transpose(1, 0, 2))
            in_maps.append({
                "keyT": keyT[b],
                "valT": valT[b],
                "qT": np.ascontiguousarray(query[b][:, c0:c0 + C].T),
                "wkT": np.ascontiguousarray(W_k[c0:c0 + C, :].T),
                "wvT": np.ascontiguousarray(W_v[c0:c0 + C, :].T),
                "woT": woT,
            })

    _last_in_maps = in_maps
    res = run_bass_kernel_spmd(
        _nc, in_maps, core_ids=list(range(8)),
        trace=bool(os.environ.get("BASS_TRACE")))
    _last_results = res

    out = np.zeros((B, NQ, E), dtype=np.float32)
    for b in range(B):
        for g in range(4):
            out[b] += res.results[b * 4 + g]["out"]
    return out


# revision 31
# speedup vs baseline: 1.0115x; 1.0115x over previous
"""MultiHeadAttention Trainium2 kernel (8 NeuronCores).

Reference computation (torch-style Linear, x @ W.T):
    k = key @ W_k.T; v = value @ W_v.T; q = query (no projection)
    scores = q @ k.T / sqrt(64) per head; attn = softmax(scores)
    out = (attn @ v) @ W_o.T

Sharding: core = (batch b, head-group g) with b in {0,1}, g in {0..3};
each core owns 4 heads of one batch. Projection weights are column-split
by head so K/V projections and attention stay core-local; the final W_o
matmul is computed as a partial sum over the core's 256 head-channels and
the 4 partials per batch are summed on host.

On-device dataflow per core (all matmuls float32r, full PE rate):
    kT[256,4096]  = W_kT.T @ keyT          (contraction over embed)
    v[4096,256]   = valueT.T @ W_vT        (+ ones column per head)
    scoresT[t,q]  = kT_h.T @ qT_h          (K=64; two heads run concurrently
                                            via tile_position rows 0/64)
    expT          = exp(scoresT / 8)       (ScalarE, from PSUM)
    outT[65,q]    = v_ext_h.T @ expT       (accumulated over 32 t-chunks;
                                            row 64 = softmax denominator)
    norm          = outT[0:64] * recip(outT[64])
    out_partial   = norm_heads.T @ W_oT    (accumulated over 4 heads)

Two-phase PSUM schedule: while the K/V stream + projections run (DMA-bound,
~93us), TWO attention sweeps (q-tile 0, both head pairs) run concurrently
from a shared 6-slot PSUM pool with single-bank score tiles, keeping ScalarE
fed. After the stream, pools are re-opened and the remaining 6 sweeps run
with double-buffered [128,1024] score tiles (higher exp efficiency).
"""

import os
import numpy as np

import concourse.bacc as bacc
import concourse.tile as tile
import concourse.mybir as mybir
from concourse.bass_utils import run_bass_kernel_spmd

F32 = mybir.dt.float32
F32R = mybir.dt.float32r
EXPF = mybir.ActivationFunctionType.Exp

B, NQ, NK, E, H, D = 2, 2048, 4096, 1024, 16, 64
HPC = 4          # heads per core
C = HPC * D      # head-channels per core (256)
TB = 256         # token block for streaming K/V projections
NTB = NK // TB   # 16
TCH = NK // 128  # 32 t-chunks for attention
QT = 512         # q tile
NJ = NQ // QT    # 4

_last_results = None
_last_in_maps = None


def _build():
    nc = bacc.Bacc("TRN2", target_bir_lowering=False, debug=False, num_devices=8)

    keyT_d = nc.dram_tensor("keyT", [E, NK], F32, kind="ExternalInput").ap()
    valT_d = nc.dram_tensor("valT", [E, NK], F32, kind="ExternalInput").ap()
    qT_d = nc.dram_tensor("qT", [C, NQ], F32, kind="ExternalInput").ap()
    wkT_d = nc.dram_tensor("wkT", [E, C], F32, kind="ExternalInput").ap()
    wvT_d = nc.dram_tensor("wvT", [E, C], F32, kind="ExternalInput").ap()
    woT_d = nc.dram_tensor("woT", [D, HPC, E], F32, kind="ExternalInput").ap()
    out_d = nc.dram_tensor("out", [NQ, E], F32, kind="ExternalOutput").ap()

    with tile.TileContext(nc) as tc:
        with (
            tc.tile_pool(name="wpool", bufs=1) as wpool,
            tc.tile_pool(name="stream", bufs=3) as stream,
            tc.tile_pool(name="big", bufs=1) as big,
            tc.tile_pool(name="expp", bufs=3) as expp,
            tc.tile_pool(name="epil", bufs=2) as epil,
            tc.tile_pool(name="normp", bufs=5) as normp,
            tc.tile_pool(name="outsb", bufs=2) as outsb,
        ):
            # ---- resident weights / q ----
            wk_sb = wpool.tile([128, 8, C], F32R)
            wv_sb = wpool.tile([128, 8, C], F32R)
            wo_sb = wpool.tile([D, HPC, E], F32R)
            q_sb = wpool.tile([128, 2, NQ], F32R)
            nc.sync.dma_start(wk_sb[:], wkT_d.rearrange("(c p) n -> p c n", p=128).bitcast(F32R))

            # ---- resident kT / v_ext ----
            kT_sb = big.tile([128, 2, NK], F32R)            # [hd%128, hd//128, t]
            vx_sb = big.tile([128, TCH, HPC, D + 1], F32R)  # [t%128, t//128, h, d|1]
            for t in range(TCH):
                nc.vector.memset(vx_sb[:, t, :, D:D + 1].bitcast(F32), 1.0)

            def emit_scores_pair(sdst_a, sdst_b, pr, t, q0):
                nc.tensor.matmul(sdst_a,
                                 kT_sb[0:64, pr, t * 128:(t + 1) * 128],
                                 q_sb[0:64, pr, q0:q0 + QT],
                                 start=True, stop=True, tile_position=(0, 0))
                nc.tensor.matmul(sdst_b,
                                 kT_sb[64:128, pr, t * 128:(t + 1) * 128],
                                 q_sb[64:128, pr, q0:q0 + QT],
                                 start=True, stop=True, tile_position=(64, 0))

            def emit_attnv(oA, oB, ex, pr, t):
                hA, hB = 2 * pr, 2 * pr + 1
                nc.tensor.matmul(oA[:], vx_sb[:, t, hA, :], ex[:, 0:QT],
                                 start=(t == 0), stop=(t == TCH - 1))
                nc.tensor.matmul(oB[:], vx_sb[:, t, hB, :], ex[:, QT:2 * QT],
                                 start=(t == 0), stop=(t == TCH - 1))

            def emit_epilogue(oT, norm_tiles):
                ocp = epil.tile([D + 1, QT], F32, tag="ocp", bufs=3, name="ocp")
                nc.vector.tensor_copy(ocp[:], oT[:])
                rc = epil.tile([1, QT], F32R, tag="recip", name="rc")
                with nc.allow_low_precision(reason="f32r recip, ~19-bit mantissa is ample"):
                    nc.vector.reciprocal(rc[0:1, :], ocp[64:65, :])
                bc = epil.tile([64, QT], F32R, tag="bcast", name="bc")
                nc.gpsimd.partition_broadcast(bc[:], rc[0:1, :])
                nm = normp.tile([64, QT], F32R, tag="norm", name="nm")
                nc.vector.tensor_mul(nm[:], ocp[0:64, :], bc[:])
                norm_tiles.append(nm)

            def emit_wo(j, norm_tiles, psw, heads=tuple(range(HPC)), accum=False,
                        wps_tag="wps"):
                # W_o partial over the given heads; accum=True accumulates
                # into the DRAM rows via SWDGE instead of overwriting.
                q0 = j * QT
                for mc in range(4):
                    osb = outsb.tile([128, E], F32, tag="osb", name="osb")
                    for et in range(2):
                        wps = psw.tile([128, QT], F32, tag=wps_tag, name="wps")
                        for i, h in enumerate(heads):
                            nc.tensor.matmul(wps[:],
                                             norm_tiles[h][:, mc * 128:(mc + 1) * 128],
                                             wo_sb[:, h, et * QT:(et + 1) * QT],
                                             start=(i == 0), stop=(i == len(heads) - 1))
                        nc.vector.tensor_copy(osb[:, et * QT:(et + 1) * QT], wps[:])
                    dst = out_d[q0 + mc * 128:q0 + (mc + 1) * 128, :]
                    if accum:
                        nc.gpsimd.dma_start(dst, osb[:], accum_op=mybir.AluOpType.add)
                    else:
                        nc.sync.dma_start(dst, osb[:])

            # ================= PHASE 1: stream + projections + j0 =================
            norm_j0 = []
            with (
                tc.tile_pool(name="pm6", bufs=6, space="PSUM") as pm6,
                tc.tile_pool(name="ps2", bufs=2, space="PSUM") as ps2,
            ):
                # two concurrent sweeps: j=0, both pairs, single-bank score tiles
                o_acc = {}
                for pr in range(2):
                    o_acc[pr] = (pm6.tile([D + 1, QT], F32, tag="p1", name="oA"),
                                 pm6.tile([D + 1, QT], F32, tag="p1", name="oB"))

                # K/V projections streamed over t blocks, interleaved with the
                # two j0 sweeps chasing the stream
                for tb in range(NTB):
                    kblk = stream.tile([128, 8, TB], F32R, tag="kblk", name="kblk")
                    vblk = stream.tile([128, 8, TB], F32R, tag="vblk", name="vblk")
                    ts0 = tb * TB
                    nc.sync.dma_start(kblk[:], keyT_d.rearrange("(c p) t -> p c t", p=128)[:, :, ts0:ts0 + TB].bitcast(F32R))
                    if tb == 0:
                        # critical-path loads for the first scores/exp: q's j0
                        # slice, then V-side weights; the rest of q after.
                        nc.sync.dma_start(q_sb[:, :, 0:QT], qT_d.rearrange("(c p) n -> p c n", p=128)[:, :, 0:QT].bitcast(F32R))
                    nc.sync.dma_start(vblk[:], valT_d.rearrange("(c p) t -> p c t", p=128)[:, :, ts0:ts0 + TB].bitcast(F32R))
                    if tb == 0:
                        nc.sync.dma_start(wv_sb[:], wvT_d.rearrange("(c p) n -> p c n", p=128).bitcast(F32R))
                        nc.sync.dma_start(q_sb[:, :, QT:NQ], qT_d.rearrange("(c p) n -> p c n", p=128)[:, :, QT:NQ].bitcast(F32R))
                    for mc in range(2):
                        kps = pm6.tile([128, TB], F32, tag="p1", name="kps")
                        for c in range(8):
                            nc.tensor.matmul(kps[:], wk_sb[:, c, mc * 128:(mc + 1) * 128],
                                             kblk[:, c, :], start=(c == 0), stop=(c == 7))
                        nc.vector.tensor_copy(kT_sb[:, mc, ts0:ts0 + TB], kps[:])
                    for t2 in range(TB // 128):
                        vps = pm6.tile([128, C], F32, tag="p1", name="vps")
                        for c in range(8):
                            nc.tensor.matmul(vps[:], vblk[:, c, t2 * 128:(t2 + 1) * 128],
                                             wv_sb[:, c, :], start=(c == 0), stop=(c == 7))
                        tg = tb * (TB // 128) + t2
                        nc.vector.tensor_copy(
                            vx_sb[:, tg, :, 0:D],
                            vps[:].rearrange("p (h d) -> p h d", h=HPC))
                    # j0 sweeps for the two t-chunks this block just produced
                    for t in (2 * tb, 2 * tb + 1):
                        for pr in range(2):
                            s1 = ps2.tile([128, QT], F32, tag="ssc", name="s1")
                            s2 = ps2.tile([128, QT], F32, tag="ssc", name="s2")
                            emit_scores_pair(s1[:], s2[:], pr, t, 0)
                            ex = expp.tile([128, 2 * QT], F32R, tag="exp", name="ex")
                            nc.scalar.activation(ex[:, 0:QT], s1[:], EXPF, scale=0.125)
                            nc.scalar.activation(ex[:, QT:2 * QT], s2[:], EXPF, scale=0.125)
                            emit_attnv(o_acc[pr][0], o_acc[pr][1], ex, pr, t)
                nc.sync.dma_start(wo_sb[:], woT_d[:].bitcast(F32R))
                for pr in range(2):
                    emit_epilogue(o_acc[pr][0], norm_j0)
                    emit_epilogue(o_acc[pr][1], norm_j0)

            # ================= PHASE 2: j1..j3 + all W_o =================
            with (
                tc.tile_pool(name="pscore", bufs=2, space="PSUM") as pscore,
                tc.tile_pool(name="pout", bufs=3, space="PSUM") as pout,
                tc.tile_pool(name="psw", bufs=1, space="PSUM") as psw,
            ):
                for j in range(1, NJ):
                    q0 = j * QT
                    norm_tiles = []
                    for pr in range(2):
                        if j == 1 and pr == 1:
                            # j0's W_o emitted after j1/pr0's sweep so the
                            # exp-feeding sweep outranks it in priority
                            emit_wo(0, norm_j0, psw)
                        oA = pout.tile([D + 1, QT], F32, tag="outp", name="oA")
                        oB = pout.tile([D + 1, QT], F32, tag="outp", name="oB")
                        for t in range(TCH):
                            sc = pscore.tile([128, 2 * QT], F32, tag="score", name="sc")
                            emit_scores_pair(sc[:, 0:QT], sc[:, QT:2 * QT], pr, t, q0)
                            ex = expp.tile([128, 2 * QT], F32R, tag="exp", name="ex")
                            nc.scalar.activation(ex[:], sc[:], EXPF, scale=0.125)
                            emit_attnv(oA, oB, ex, pr, t)
                        emit_epilogue(oA, norm_tiles)
                        emit_epilogue(oB, norm_tiles)
                        if j == NJ - 1 and pr == 0:
                            # last q-tile: flush pair 0's W_o early so only
                            # pair 1's half remains after the final sweep
                            emit_wo(j, norm_tiles, psw, heads=(0, 1))
                    if j == NJ - 1:
                        emit_wo(j, norm_tiles, psw, heads=(2, 3), accum=True)
                    else:
                        emit_wo(j, norm_tiles, psw)

    nc.compile()
    return nc


_nc = None


def kernel(query, key, value, W_k, W_v, W_o):
    global _nc, _last_results, _last_in_maps
    if _nc is None:
        _nc = _build()

    query = np.asarray(query, dtype=np.float32)
    key = np.asarray(key, dtype=np.float32)
    value = np.asarray(value, dtype=np.float32)
    W_k = np.asarray(W_k, dtype=np.float32)
    W_v = np.asarray(W_v, dtype=np.float32)
    W_o = np.asarray(W_o, dtype=np.float32)

    keyT = [np.ascontiguousarray(key[b].T) for b in range(B)]
    valT = [np.ascontiguousarray(value[b].T) for b in range(B)]

    in_maps = []
    for b in range(B):
        for g in range(4):
            c0 = g * C
            woT = np.ascontiguousarray(
                W_o[:, c0:c0 + C].T.reshape(HPC, D, E).transpose(1, 0, 2))
            in_maps.append({
                "keyT": keyT[b],
                "valT": valT[b],
                "qT": np.ascontiguousarray(query[b][:, c0:c0 + C].T),
                "wkT": np.ascontiguousarray(W_k[c0:c0 + C, :].T),
                "wvT": np.ascontiguousarray(W_v[c0:c0 + C, :].T),
                "woT": woT,
            })

    _last_in_maps = in_maps
    res = run_bass_kernel_spmd(
        _nc, in_maps, core_ids=list(range(8)),
        trace=bool(os.environ.get("BASS_TRACE")))
    _last_results = res

    out = np.zeros((B, NQ, E), dtype=np.float32)
    for b in range(B):
        for g in range(4):
            out[b] += res.results[b * 4 + g]["out"]
    return out
